# revision 1
# baseline (speedup 1.0000x reference)
"""AdaptiveSpectrumLayer Trainium2 kernel (8-core data-parallel), v2.

Structure (vs v1):
  * rfft/irfft over H=512 are DFT matmuls (bf16).  The 257 rfft bins pack
    4 M-tiles of 128: [re 0..127][re 128..255][re256 @row0; im 1..127]
    [im 128..255] — re256 rides the structurally-zero im[0] row, so the
    separate M=1 Nyquist matmul disappears (a 1-row fixup restores
    mag[0] = |re[0]| for the gate features).
  * Gate y = feats @ A is computed TRANSPOSED: out[col, m=257] with the
    feature axis contracted on partitions (7 K-tiles x 257-free matmuls),
    so the m=256 row no longer costs a full M=1 pass and softmax over m
    becomes a free-axis op: exp with accum_out gives the row sum for
    free, reciprocal_approx_fast + a per-partition scalar multiply
    normalizes w in-place.  Normalized w is transposed back to [n, col]
    with cheap 128-wide PE transposes; deferred-normalization epilogue
    from v1 is gone (plain PSUM->SBUF copies).
  * nc.vector.reciprocal (~6 cycles/elem on HW) is replaced everywhere
    by RECIPROCAL_APPROX_FAST custom-DVE ops (~51 ULP, 1 pass).
  * Elementwise work is spread across DVE / Act / Pool by measured cost;
    fft tiles (Tc) are bf16 so the u = fft*w multiplies hit the DVE
    2x 16-bit mode.
Sharded batch 128 -> 8 cores x 16.  Gate matmuls in float32r (free dim
257 >= 256 keeps full rate), DFT matmuls bf16.
"""

import numpy as np

B, H, F = 128, 512, 64
HID = 16
NF = H // 2 + 1          # 257
NCORES = 8
BL = B // NCORES         # 16 batch per core
BF = BL * F              # 1024 free columns per core
P = 128
CH = 512                 # free-dim chunk (8 batches x 64)
NCH = BF // CH           # 2
NQ = CH // P             # 4 col-subtiles of 128 per chunk


def _build_constants(W_proj, b_proj, W_gate, b_gate):
    W_proj = np.asarray(W_proj, np.float64)
    b_proj = np.asarray(b_proj, np.float64)
    W_gate = np.asarray(W_gate, np.float64)
    b_gate = np.asarray(b_gate, np.float64)

    Wg = W_gate.reshape(NF, NF, HID)                      # [m, n, h]
    A = np.einsum("nch,mnh->ncm", W_proj, Wg)             # (257, 3, 257)
    bias_eff = b_gate + np.einsum("nh,mnh->m", b_proj, Wg)

    h = np.arange(H)
    n = np.arange(NF)
    ang = 2.0 * np.pi * np.outer(h, n) / H                # (512, 257)
    Cf = np.cos(ang)
    Sf = -np.sin(ang)

    # forward DFT weights: (512 h, 4*128 packed outputs); tile2 col0 now
    # carries re256 (the im[0] row is structurally zero).
    Wf = np.concatenate(
        [
            Cf[:, 0:128],
            Cf[:, 128:256],
            np.concatenate([Cf[:, 256:257], Sf[:, 1:128]], axis=1),
            Sf[:, 128:256],
        ],
        axis=1,
    ).astype(np.float32)                                  # (512, 512)

    # gate rhs tiles (transposed gate): 8 K-tiles x 128 x 257
    # (tiles 6/7 are single-row: the Nyquist mag/cos features)
    Ap = np.zeros((8, P, NF), dtype=np.float32)
    Ap[0] = A[0:128, 0, :]
    Ap[1] = A[128:256, 0, :]
    Ap[2] = A[0:128, 1, :]
    Ap[2, 0] = bias_eff          # sin[0] is always 0 -> row reused for bias
    Ap[3] = A[128:256, 1, :]
    Ap[4] = A[0:128, 2, :]
    Ap[5] = A[128:256, 2, :]
    Ap[6, 0] = A[256, 0, :]      # mag256 row
    Ap[6, 1] = A[256, 2, :]      # cos256 row (K=2 tile with mag256)

    # inverse DFT weights: 4 K-tiles x 128 x 512
    cinv = np.cos(ang)
    sinv = np.sin(ang)
    cn = np.full(NF, 2.0)
    cn[0] = 1.0
    cn[256] = 1.0
    Ci = cinv * cn[None, :] / H                           # (512, 257)
    Si = (-2.0 / H) * sinv
    Wi = np.zeros((4, P, H), dtype=np.float32)
    Wi[0] = Ci[:, 0:128].T
    Wi[1] = Ci[:, 128:256].T
    Wi[2, 0] = Ci[:, 256]                                 # Nyquist row
    Wi[2, 1:128] = Si[:, 1:128].T
    Wi[3] = Si[:, 128:256].T

    import ml_dtypes  # noqa
    Wf_p = np.ascontiguousarray(
        Wf.reshape(4, P, 512).transpose(1, 0, 2)).astype(ml_dtypes.bfloat16)
    Ap_p = np.ascontiguousarray(
        Ap.transpose(1, 0, 2)).astype(ml_dtypes.bfloat16)  # (P, 8, 257) bf16
    Wi_p = np.ascontiguousarray(
        Wi.transpose(1, 0, 2)).astype(ml_dtypes.bfloat16)  # (P, 4, 512) bf16
    eye = np.eye(P, dtype=ml_dtypes.bfloat16)              # (P, 128)
    return Wf_p, Ap_p, Wi_p, eye


def _build_graph(reps=1):
    from contextlib import ExitStack

    import concourse.bass as bass
    import concourse.tile as tile
    from concourse import bacc, mybir
    from concourse.dve_ops import RECIP_APPROX_FAST_CONSTS, RECIPROCAL_APPROX_FAST

    F32 = mybir.dt.float32
    FR = mybir.dt.float32r
    BF16 = mybir.dt.bfloat16
    AF = mybir.ActivationFunctionType
    RC = RECIP_APPROX_FAST_CONSTS

    nc = bacc.Bacc(
        "TRN2",
        target_bir_lowering=False,
        debug=False,
        num_devices=NCORES,
    )

    def recip_fast(out_ap, in_ap):
        # raw emit: skips the f32-dtype assert so float32r tiles (same bit
        # layout) are accepted
        nc.vector._custom_dve(
            RECIPROCAL_APPROX_FAST, out=out_ap, in0=in_ap,
            s0=RC["s0"], s1=RC["s1"], imm2=RC["imm2"],
        )

    # all DRAM layouts are partition-major so every DMA is fully contiguous
    x_ext = nc.dram_tensor("x", [4, P, BL, F], BF16, kind="ExternalInput").ap()
    wf_ext = nc.dram_tensor("wf", [P, 4, 512], BF16, kind="ExternalInput").ap()
    ap_ext = nc.dram_tensor("apk", [P, 8, NF], BF16, kind="ExternalInput").ap()
    wi_ext = nc.dram_tensor("wi", [P, 4, H], BF16, kind="ExternalInput").ap()
    eye_ext = nc.dram_tensor("eye", [P, P], BF16, kind="ExternalInput").ap()
    out_ext = nc.dram_tensor("out", [4, P, BL, F], F32, kind="ExternalOutput").ap()

    with tile.TileContext(nc) as tc, ExitStack() as ctx:
        const = ctx.enter_context(tc.tile_pool(name="const", bufs=1))
        tpool = ctx.enter_context(tc.tile_pool(name="reim", bufs=1))
        fpool = ctx.enter_context(tc.tile_pool(name="feats", bufs=1))
        spool = ctx.enter_context(tc.tile_pool(name="small", bufs=1))
        wpool = ctx.enter_context(tc.tile_pool(name="work", bufs=1))
        opool = ctx.enter_context(tc.tile_pool(name="outs", bufs=1))
        psmm = ctx.enter_context(tc.tile_pool(name="psmm", bufs=3, space="PSUM"))
        psy = ctx.enter_context(tc.tile_pool(name="psy", bufs=3, space="PSUM"))
        pswt = ctx.enter_context(tc.tile_pool(name="pswt", bufs=1, space="PSUM"))
        pswn = ctx.enter_context(tc.tile_pool(name="pswn", bufs=1, space="PSUM"))

        # ---- constants / input DMA (all contiguous)
        wf_sb = const.tile([P, 4, 512], BF16, tag="wf", name="wf")
        for k in range(4):
            [nc.sync, nc.gpsimd][k % 2].dma_start(wf_sb[:, k, :], wf_ext[:, k, :])
        ap_sb = const.tile([P, 8, NF], BF16, tag="apk", name="apk")
        wi_sb = const.tile([P, 4, H], BF16, tag="wi", name="wi")
        eye_sb = const.tile([P, P], BF16, tag="eye", name="eye")
        onesr_f32 = const.tile([1, P + BF], F32, tag="onesr_f32", name="onesr_f32")
        nc.vector.memset(onesr_f32[:], 1.0)
        warm = const.tile([1, 8], F32, tag="warm", name="warm")
        nc.scalar.activation(warm[:], onesr_f32[0:1, 0:8], func=AF.Sqrt)

        xpool = ctx.enter_context(tc.tile_pool(name="xin", bufs=2))

        perm = [0, 2, 1, 3]

        def emit_head():
            """x DMA + forward DFTs + features for one rep; returns state."""
            st = {}
            # input DMA rides the SP queue only; output DMAs use gpsimd's,
            # so next-rep input never queues behind this rep's output
            x_sb = xpool.tile([P, 4, BL, F], BF16, tag="x", name="x")
            for k in range(4):
                nc.sync.dma_start(x_sb[:, k, :, :], x_ext[k][:, :, :])
            # T layout: j = [re_a, im_a(re256@row0), re_b, im_b]
            # bufs=2: this rep's fwd copies don't wait on prior u-muls
            Tc = [tpool.tile([P, 4, CH], BF16, tag=f"tc{c}", name=f"tc{c}", bufs=2)
                  for c in range(NCH)]
            mags, scs, mg256s, cs256s = {}, {}, {}, {}
            st.update(x_sb=x_sb, Tc=Tc, mags=mags, scs=scs,
                      mg256s=mg256s, cs256s=cs256s)

            # ===== stages 1+2, interleaved per chunk: fwd DFT c, feats c,
            # fwd DFT c+1, feats c+1.  Emitting feats c0 before chunk 1's
            # PSUM->SBUF copies keeps the in-order Act/DVE queues from
            # blocking chunk 0's ready feature ops behind copies that wait
            # on fwd1.  (PE order is unchanged: fwd0, fwd1, gates...)
            def emit_fwd(c):
                bsl = slice(c * (CH // F), (c + 1) * (CH // F))
                # m-tile order (0,2,1,3) finishes group a (re_a, im_a)
                # first so its feature chain starts mid-fwd
                for mt in (0, 2, 1, 3):
                    ps = psmm.tile([P, CH], F32, tag="mm", name="mm")
                    for k in range(4):
                        nc.tensor.matmul(
                            ps[:],
                            wf_sb[:, k, mt * P:(mt + 1) * P],
                            x_sb[:, k, bsl, :],
                            start=(k == 0),
                            stop=(k == 3),
                        )
                    if mt in (0, 2):
                        nc.vector.tensor_copy(Tc[c][:, perm[mt], :], ps[:])
                    else:
                        nc.scalar.activation(Tc[c][:, perm[mt], :], ps[:], func=AF.Copy)

            def emit_feats(c):
                mag = fpool.tile([P, 2, CH], BF16, tag=f"mag_{c}", name=f"mag_{c}")
                sc = fpool.tile([P, 4, CH], BF16, tag=f"sc_{c}", name=f"sc_{c}")
                mg256 = spool.tile([2, CH], BF16, tag=f"mg256_{c}", name=f"mg256_{c}")
                ri256 = spool.tile([1, CH], BF16, tag=f"ri256_{c}", name=f"ri256_{c}")
                cs256 = spool.tile([1, CH], BF16, tag=f"cs256_{c}", name=f"cs256_{c}")
                ssq = wpool.tile([P, 4, CH], BF16, tag=f"ssq{c}", name=f"ssq{c}")
                sq = wpool.tile([P, 2, CH], BF16, tag=f"sq{c}", name=f"sq{c}")
                rinv = wpool.tile([P, 2, CH], BF16, tag=f"rinv{c}", name=f"rinv{c}")
                # nyquist feats: mag256 = |re256|, cos256 = re256/|re256|
                # = sign(re256) -- two Act ops, no reciprocal needed
                nc.scalar.activation(mg256[0:1, :], Tc[c][0:1, 1, :], func=AF.Abs)
                nc.scalar.activation(cs256[:], Tc[c][0:1, 1, :], func=AF.Sign)
                # compute engines cannot write partition 1; a 1KB SBUF->SBUF
                # DMA stacks the sign row under the magnitude row so the
                # gate's two Nyquist K=1 matmuls merge into one K=2
                nc.sync.dma_start(mg256[1:2, :], cs256[:])
                # column-halved chains: the first half's sin/cos land ~1.5us
                # earlier so gate q0/q1 start while q2/q3 features compute
                HH = CH // 2
                for g in range(2):
                    js = slice(2 * g, 2 * g + 2)
                    for h in range(2):
                        hs = slice(h * HH, (h + 1) * HH)
                        with nc.allow_low_precision(reason="bf16 features"):
                            nc.vector.tensor_mul(ssq[:, js, hs], Tc[c][:, js, hs],
                                                 Tc[c][:, js, hs])
                            nc.gpsimd.tensor_add(sq[:, g, hs], ssq[:, 2 * g, hs],
                                                 ssq[:, 2 * g + 1, hs])
                            if g == 0:
                                # row0: mag[0] = |re0| (im0 slot carries re256)
                                nc.vector.tensor_mul(sq[0:1, 0, hs],
                                                     Tc[c][0:1, 0, hs],
                                                     Tc[c][0:1, 0, hs])
                            nc.scalar.activation(mag[:, g, hs], sq[:, g, hs],
                                                 func=AF.Sqrt)
                            recip_fast(rinv[:, g, hs], mag[:, g, hs])
                            nc.vector.tensor_mul(
                                sc[:, js, hs],
                                Tc[c][:, js, hs],
                                rinv[:, g, None, hs].broadcast_to([P, 2, HH]),
                            )
                # bias rides the always-zero sin[0] feature row (sc j=1 row0)
                nc.gpsimd.tensor_copy(sc[0:1, 1, :], onesr_f32[0:1, P:P + CH])
                mags[c], scs[c], mg256s[c], cs256s[c] = mag, sc, mg256, cs256

            for c in range(NCH):
                emit_fwd(c)
                emit_feats(c)
            return st

        def emit_gates(st):
            # ===== gate + softmax for both chunks (PE dense)
            wqs = {}
            for c in range(NCH):
                mag, sc = st["mags"][c], st["scs"][c]
                mg256, cs256 = st["mg256s"][c], st["cs256s"][c]
                wq = []
                for q in range(NQ):
                    qsl = slice(q * P, (q + 1) * P)
                    # group-a features first: the gate can start before the
                    # group-b feature chain finishes
                    klist = [
                        (mag[:, 0, qsl], 0),
                        (sc[:, 1, qsl], 2),
                        (sc[:, 0, qsl], 4),
                        (mg256[0:2, qsl], 6),
                        (mag[:, 1, qsl], 1),
                        (sc[:, 3, qsl], 3),
                        (sc[:, 2, qsl], 5),
                    ]
                    ps_y = psy.tile([P, NF], F32, tag="y", name="psy")
                    for i, (lhsT, kt) in enumerate(klist):
                        nc.tensor.matmul(
                            ps_y[:],
                            lhsT,
                            ap_sb[0:lhsT.partition_size(), kt, :],
                            start=(i == 0),
                            stop=(i == len(klist) - 1),
                        )
                    # e = exp(silu(y)) via tanh; accum_out gives the row sum
                    th = wpool.tile([P, NF], F32, tag="th", name="th", bufs=6)
                    nc.scalar.activation(th[:], ps_y[:], func=AF.Tanh, scale=0.5)
                    ysw = wpool.tile([P, NF], F32, tag="ysw", name="ysw", bufs=6)
                    nc.vector.scalar_tensor_tensor(
                        out=ysw[:], in0=th[:], scalar=1.0, in1=ps_y[:],
                        op0=mybir.AluOpType.add, op1=mybir.AluOpType.mult,
                    )
                    e = wpool.tile([P, NF], BF16, tag="e", name="e", bufs=6)
                    s = spool.tile([P, 1], F32, tag=f"s_{c}_{q}", name=f"s_{c}_{q}")
                    nc.scalar.activation(e[:], ysw[:], func=AF.Exp, scale=0.5,
                                         accum_out=s[:])
                    srec = spool.tile([P, 1], F32, tag=f"sr_{c}_{q}",
                                      name=f"sr_{c}_{q}")
                    recip_fast(srec[:], s[:])
                    w = wpool.tile([P, NF], BF16, tag="w", name="w", bufs=8)
                    with nc.allow_low_precision(reason="softmax normalize bf16"):
                        nc.gpsimd.tensor_scalar_mul(w[:], e[:], srec[:])
                    wq.append(w)
                wqs[c] = wq
            st["wqs"] = wqs

        def emit_trs_u(st):
            # ===== per chunk: w-transpose -> u
            Us = {}
            Tc = st["Tc"]
            for c in range(NCH):
                wT = pswt.tile([P, 2, CH], BF16, tag="wt", name="wt")
                wnT = pswn.tile([1, CH], BF16, tag="wnt", name="wnt")
                for q in range(NQ):
                    qsl = slice(q * P, (q + 1) * P)
                    w = st["wqs"][c][q]
                    nc.tensor.transpose(wT[:, 0, qsl], w[:, 0:P], eye_sb[:])
                    nc.tensor.transpose(wT[:, 1, qsl], w[:, P:2 * P], eye_sb[:])
                    nc.tensor.transpose(wnT[0:1, qsl], w[:, 2 * P:NF], eye_sb[:])

                Ua = fpool.tile([P, 2, CH], BF16, tag=f"ua_{c}", name=f"ua_{c}")
                Ub = fpool.tile([P, 2, CH], BF16, tag=f"ub_{c}", name=f"ub_{c}")
                with nc.allow_low_precision(reason="u bf16"):
                    nc.vector.tensor_mul(
                        Ua[:], Tc[c][:, 0:2, :],
                        wT[:, 0, None, :].broadcast_to([P, 2, CH]),
                    )
                    # nyquist u row: re256 * w256 rides the im[0] slot
                    nc.vector.tensor_mul(Ua[0:1, 1, :], Tc[c][0:1, 1, :], wnT[:])
                    nc.vector.tensor_mul(
                        Ub[:], Tc[c][:, 2:4, :],
                        wT[:, 1, None, :].broadcast_to([P, 2, CH]),
                    )
                Us[c] = (Ua, Ub)
            st["Us"] = Us

        def emit_inv(st):
            # ===== inverse DFTs + output DMA
            zout = [
                opool.tile([P, NCH, CH], F32, tag=f"z{m}", name=f"z{m}")
                for m in range(4)
            ]
            for c in range(NCH):
                Ua, Ub = st["Us"][c]
                U = [Ua[:, 0, :], Ub[:, 0, :], Ua[:, 1, :], Ub[:, 1, :]]
                korder = [0, 2, 1, 3]
                for mt in range(4):
                    ps = psmm.tile([P, CH], F32, tag="mm", name="psz")
                    for j, k in enumerate(korder):
                        nc.tensor.matmul(
                            ps[:],
                            wi_sb[:, k, mt * P:(mt + 1) * P],
                            U[k],
                            start=(j == 0),
                            stop=(j == 3),
                        )
                    # Act-triggered DMAs ride HWDGE (no engine time); the
                    # DVE-copied tiles keep gpsimd SWDGE triggers
                    if mt in (0, 2):
                        nc.scalar.activation(zout[mt][:, c, :], ps[:], func=AF.Copy)
                        deng = nc.scalar
                    else:
                        nc.vector.tensor_copy(zout[mt][:, c, :], ps[:])
                        deng = nc.gpsimd
                    deng.dma_start(
                        out_ext[mt][:, c * (CH // F):(c + 1) * (CH // F), :],
                        zout[mt][:, c, :].rearrange("p (b f) -> p b f", f=F),
                    )

        # ===== software pipeline across reps: the next rep's fwd DFT is
        # emitted between this rep's transposes and inverse so the PE's
        # u-mult and rep-boundary waits are filled with fwd matmuls
        st = emit_head()
        nc.sync.dma_start(ap_sb[:], ap_ext)
        nc.gpsimd.dma_start(wi_sb[:], wi_ext)
        nc.gpsimd.dma_start(eye_sb[:], eye_ext)
        for r in range(reps):
            emit_gates(st)
            emit_trs_u(st)
            nxt = emit_head() if r + 1 < reps else None
            emit_inv(st)
            st = nxt

    nc.compile()
    return nc


_CACHE = {}


def _bf16():
    import ml_dtypes
    return ml_dtypes.bfloat16


def _pack_in_maps(inputs):
    Wf, Ap, Wi, eye = _build_constants(
        inputs["W_proj"], inputs["b_proj"], inputs["W_gate"], inputs["b_gate"]
    )
    x = np.ascontiguousarray(np.asarray(inputs["x"], np.float32))
    return [
        {
            # (BL,H,F) -> (4,P,BL,F): h-tile-major, partition-contiguous
            "x": np.ascontiguousarray(
                x[c * BL:(c + 1) * BL].transpose(1, 0, 2).reshape(4, P, BL, F)
            ).astype(_bf16()),
            "wf": Wf,
            "apk": Ap,
            "wi": Wi,
            "eye": eye,
        }
        for c in range(NCORES)
    ]


def _run(inputs, trace=False):
    from concourse.bass_utils import run_bass_kernel_spmd

    if "graph" not in _CACHE:
        _CACHE["graph"] = _build_graph()
    nc = _CACHE["graph"]
    in_maps = _pack_in_maps(inputs)
    res = run_bass_kernel_spmd(nc, in_maps, core_ids=list(range(NCORES)), trace=trace)
    # (4,P,BL,F) -> (BL,H,F)
    out = np.concatenate(
        [r["out"].transpose(2, 0, 1, 3).reshape(BL, H, F) for r in res.results],
        axis=0,
    )
    return out.astype(np.float32), res


def kernel(**inputs):
    out, _ = _run(inputs, trace=False)
    return out


def _make_exec(nc):
    """Build a jit-cached 8-core executor for a compiled Bacc graph,
    replicating bass2jax.run_bass_via_pjrt's multi-core path but reusable
    across calls (for timing)."""
    import jax
    import numpy as np
    from jax.sharding import Mesh, PartitionSpec
    from jax.experimental.shard_map import shard_map
    from concourse import mybir
    from concourse.bass2jax import _bass_exec_p, install_neuronx_cc_hook

    install_neuronx_cc_hook()
    from concourse.bass2jax import partition_id_tensor

    n_cores = NCORES
    pid_name = nc.partition_id_tensor.name if nc.partition_id_tensor else None
    in_names, out_names, out_avals, zero_outs = [], [], [], []
    for alloc in nc.m.functions[0].allocations:
        if not isinstance(alloc, mybir.MemoryLocationSet):
            continue
        name = alloc.memorylocations[0].name
        if alloc.kind == "ExternalInput":
            if name != pid_name:
                in_names.append(name)
        elif alloc.kind == "ExternalOutput":
            out_names.append(name)
            shape = tuple(alloc.tensor_shape)
            dtype = mybir.dt.np(alloc.dtype)
            out_avals.append(jax.core.ShapedArray(shape, dtype))
            zero_outs.append(np.zeros(shape, dtype))
    n_params = len(in_names)
    all_names = in_names + out_names
    if pid_name is not None:
        all_names = all_names + [pid_name]

    def _body(*args):
        operands = list(args)
        if pid_name is not None:
            operands.append(partition_id_tensor())
        outs = _bass_exec_p.bind(
            *operands,
            out_avals=tuple(out_avals),
            in_names=tuple(all_names),
            out_names=tuple(out_names),
            lowering_input_output_aliases=(),
            sim_require_finite=True,
            sim_require_nnan=True,
            nc=nc,
        )
        return tuple(outs)

    devices = jax.devices()[:n_cores]
    mesh = Mesh(np.asarray(devices), ("core",))
    n_all = n_params + len(out_names)
    fn = jax.jit(
        shard_map(
            _body,
            mesh=mesh,
            in_specs=(PartitionSpec("core"),) * n_all,
            out_specs=(PartitionSpec("core"),) * len(out_names),
            check_rep=False,
        ),
        keep_unused=True,
    )

    def pack(in_maps):
        concat = [
            np.concatenate([np.asarray(in_maps[c][k]) for c in range(n_cores)], axis=0)
            for k in in_names
        ]
        concat += [
            np.zeros((n_cores * z.shape[0], *z.shape[1:]), z.dtype) for z in zero_outs
        ]
        return [jax.device_put(a) for a in concat]

    return fn, pack, out_names, out_avals

